# revision 76
# baseline (speedup 1.0000x reference)
"""Trainium2 Bass kernel for the 5x5-neighborhood min-L1 loss (nn_NNLoss).

Computation (faithful to the reference):
    gt_pad = pad(ground_truth, rows by nw//2, cols by nh//2, value=-10000)
    norms[b,h,w,s] = sum_c |gt_pad[b,c,h+di,w+dj] - pred[b,c,h,w]|
                     for s=(di,dj), di in range(nh), dj in range(nw)
    loss = mean over (b,h,w) of min_s norms

Sharding: pure data parallel over the batch dim: 16 images -> 2 per core
across 8 NeuronCores.  Each core returns per-partition partial sums
[128,1]; the host adds them up and divides (the scalar "all-reduce").

Per-core layout (single row-block, 2 rows per partition):
  - partition p holds image rows {2p, 2p+1} (sub-row s in {0,1}); free
    dim is [q=(img,chan), s, w].  Every HBM load uses 2KB contiguous
    descriptors (2 rows per partition) and the whole H=256 fits one
    partition block.  Loads are split per image so the leading compute
    can start on the first image's half ~4us earlier.
  - ground_truth is loaded ONCE via SWDGE dmas that cast f32->bf16 in
    flight; the nh row shifts decompose into partition shifts k in
    {-1,0,+1} plus a sub-row select s'.  The shifted copies are built
    by the otherwise-idle TensorEngine: matmul against a shifted
    identity (exact for bf16), then one ACT cast PSUM->SBUF.  (DMA
    alternatives measured far worse: SBUF->SBUF copies crawl at 5-23
    GB/s, and a third concurrent HBM load stream collapses the DMA
    subsystem to ~12 GB/s.)  The boundary partition left empty by the
    shift matmul is patched to +10000 by a tiny one-partition DMA.
  - NO pad values are materialized: out-of-range column shifts are
    excluded from the running min by restricting the min-update APs to
    the valid w range, and out-of-range rows lose every min because the
    boundary partitions hold +10000 (real sums are < ~30).
  - per (di, s) unit: one wide sub (DVE, all nw column shifts via an
    overlapping-window AP at 2x bf16) -> |.| in place (ACT, ~1 elem/
    cycle) -> channel sum (2 DVE adds) landing in a PAIR tile shared by
    both sub-rows of the di.  Once both halves land, ONE w-restricted
    running-min update per (di, g) covers both sub-rows (24 ops instead
    of 48 -- each DVE op pays ~200ns init+semaphore overhead), into a
    shared m tile [i][s][w], reduced once at the end.
  - the [128,1] per-partition partials are collapsed to a single [1,1]
    scalar by a ones-vector matmul on the PE before the output dma: a
    [128,1] dma is 128 4-byte descriptors whose completion receipt
    burned ~7us of pure tail; the scalar dma is one descriptor (~2us).
  - SBUF allocation order is performance-critical: inserting one tiny
    (4B) tile in the middle of the allocation sequence shifted the hot
    tiles' addresses and slowed EVERY DVE op by ~20%.  Small scratch
    tiles are therefore allocated last.
  - engine balance: DVE ~102us busy (subs 41 + adds 28 + mins 17, all
    at the 2x_1p theoretical floor), ACT ~89us (abs 66 + casts),
    PE/Pool/Sync light.  Wall ~110us = ~7us runtime preamble + ~10us
    HBM-bound load ramp + DVE stream + ~4us scalar-out/teardown tail.
"""

import os

# The execution path needs the axon PJRT platform; a harness that pins
# JAX_PLATFORMS=cpu would hide the NeuronCores from jax.
if "axon" not in os.environ.get("JAX_PLATFORMS", "axon"):
    os.environ.pop("JAX_PLATFORMS", None)

import numpy as np

B, C, H, W = 16, 3, 256, 256
N_CORES = 8
IPC = B // N_CORES  # images per core
PAD_BIG = 10000.0  # stand-in for the reference's pad: never wins the min

_BUILD_CACHE = {}
LAST_EXEC_NS = [None]  # exec_time_ns of the last traced run (for test.py)
LAST_RES = [None]  # full BassKernelResults of the last run (for analysis)


def _build(nh, nw):
    """Trace the Bass/Tile program for one core. Returns the Bass object."""
    from contextlib import ExitStack

    import concourse.bacc as bacc
    import concourse.bass as bass  # noqa: F401
    import concourse.tile as tile
    from concourse import mybir
    from concourse.alu_op_type import AluOpType

    f32 = mybir.dt.float32
    # bf16, not fp16: the DVE's 2x tensor_tensor packing mode only has
    # uops for bf16 (fp16 measured at 1x on HW)
    f16 = mybir.dt.bfloat16
    Abs = mybir.ActivationFunctionType.Abs
    Copy = mybir.ActivationFunctionType.Copy

    # Faithful to the reference's crossed pad/shift pairing:
    #   row shifts   di in range(nh), offset d  = di - nw//2
    #   col shifts   g  in range(nw), offset    = g  - nh//2
    H_PAD = nw // 2
    W_PAD = nh // 2
    NDI, G = nh, nw
    S = 2  # rows packed per partition
    assert H == 128 * S
    Q = C * IPC  # fused (img, chan) chunks: 6
    SW = S * W  # 512
    FDW = Q * SW  # 3072: data columns of the packed tiles
    MARG = W_PAD  # margin columns so the window AP stays in-bounds
    GQW = G * Q * W  # 7680: one (di, s) diff tensor [g][q][w]
    GIW = G * IPC * W  # 2560: one (di, s) channel-summed tensor [g][i][w]
    IW = IPC * SW  # 1024: running-min tile [i][s][w]

    # (di, s) -> (partition shift k, source sub-row s'): the target row
    # 2p + s + (di - H_PAD) lives at partition p + k, sub-row s'
    def shift_of(di, s):
        idx = s + di - H_PAD
        return idx // S, idx % S

    all_units = [(di, s) for di in range(NDI) for s in range(S)]
    ks_needed = sorted({shift_of(*u)[0] for u in all_units})
    # process units that only need the unshifted tile first (the PE
    # builds the shifted tiles while the first subs run), and within
    # that order by row shift so each di's pair completes early: the
    # min updates run once per (di, g) over BOTH sub-rows
    di_order = sorted(
        range(NDI), key=lambda di: max(abs(shift_of(di, s)[0]) for s in (0, 1))
    )
    units = [u for di in di_order for u in ((di, 0), (di, 1))
             if shift_of(*u)[0] == 0]
    units += [u for di in di_order for u in ((di, 0), (di, 1))
              if shift_of(*u)[0] != 0]

    # valid output-w range for column shift g (shifts reading outside the
    # row are excluded from the min -- the reference's pad value loses
    # every min it enters, so exclusion is equivalent)
    def wrange(g):
        lo = max(0, W_PAD - g)
        hi = W + min(0, W_PAD - g)
        return lo, hi

    # Bacc (not raw Bass): its compile() splits multi-wait instructions
    # (TRN2 allows at most one sync wait per instruction) among other
    # required lowerings.
    nc = bacc.Bacc(
        "TRN2", target_bir_lowering=False, debug=False, num_swdge_queues=2
    )
    pred_d = nc.dram_tensor("predicted", [IPC, C, H, W], f32, kind="ExternalInput")
    gt_d = nc.dram_tensor("ground_truth", [IPC, C, H, W], f32, kind="ExternalInput")
    # stacked shifted identities [k-index, 128, 128] for the PE-based
    # partition shifts (lhsT[k, p] = 1 iff k = p + shift)
    n_eyes = len([k for k in ks_needed if k != 0])
    eye_d = nc.dram_tensor("shifteye", [128, n_eyes * 128], f16, kind="ExternalInput")
    out_d = nc.dram_tensor("partials", [1, 1], f32, kind="ExternalOutput")

    import bass_rust as _br

    def strided(ap, levels, extra_offset=0):
        """Hand-built free-dim AP on an existing [128, N] view (keeps the
        partition level and base offset)."""
        c = ap.copy()
        c.ap = _br.VecI64Pair([list(ap.ap[0])] + [list(l) for l in levels])
        if extra_offset:
            c.offset = c.offset + extra_offset
        return c

    with tile.TileContext(nc) as tc, ExitStack() as ctx:
        g_pool = ctx.enter_context(tc.tile_pool(name="gt", bufs=1))
        p_pool = ctx.enter_context(tc.tile_pool(name="pred", bufs=1))
        d_pool = ctx.enter_context(tc.tile_pool(name="d", bufs=4))
        s_pool = ctx.enter_context(tc.tile_pool(name="s", bufs=3))
        m_pool = ctx.enter_context(tc.tile_pool(name="m", bufs=1))
        r_pool = ctx.enter_context(tc.tile_pool(name="r", bufs=1))

        # ---- ground truth: SWDGE dmas, f32->bf16 cast in flight, 2KB
        # descriptors (2 contiguous rows per partition).  Both input
        # loads together read 3.1MB -- the ~9us load phase is HBM-bound
        # either way, and the in-flight cast avoids a separate cast op
        # (HWDGE + DVE casts measured net-worse: the casts tax the
        # critical DVE more than the earlier landing saves) ----
        gt_t = {}
        gt_t[0] = g_pool.tile(
            [128, MARG + FDW + MARG], f16, tag="gt0", name="gt0"
        )
        nc.gpsimd.memset(gt_t[0][:, 0:MARG], PAD_BIG)
        nc.gpsimd.memset(gt_t[0][:, MARG + FDW :], PAD_BIG)
        # three dmas with progressively larger q-chunks: the first (one
        # channel, 128 descriptors = ~1.4us of Q7 emission) lands ~5us
        # before the whole tensor, letting the leading units' subs start
        # channel-granular while the rest streams in
        HF = C * SW  # columns per image in the packed free dim
        gt_view = gt_d.ap().rearrange("i c (p s) w -> p (i c) (s w)", s=S)
        CH = H * W  # elements per (i, c) chunk in DRAM
        Q_CHUNKS = [(0, 1), (1, C), (C, Q)]
        for q0, q1 in Q_CHUNKS:
            nq = q1 - q0
            nc.gpsimd.dma_start(
                gt_t[0][:, MARG + q0 * SW : MARG + q1 * SW].rearrange(
                    "p (q x) -> p q x", q=nq
                ),
                strided(gt_view, [[CH, nq], [1, SW]], q0 * CH),
            )

        # ---- predicted: HWDGE f32 loads + ACT casts, split by image so
        # the image-0 half is ready ~4us earlier for the leading subs ----
        p_stage = p_pool.tile([128, FDW], f32, tag="p_stage", name="p_stage")
        pred_t = p_pool.tile([128, FDW], f16, tag="pred", name="pred")
        for i in range(IPC):
            nc.scalar.dma_start(
                p_stage[:, i * HF : (i + 1) * HF].rearrange(
                    "p (q x) -> p q x", q=C
                ),
                pred_d.ap()[i : i + 1].rearrange("i c (p s) w -> p (i c) (s w)", s=S),
            )
            if i == 0:
                # image-0 cast on DVE (2x_2p, 0.86us, while DVE idles in
                # the load ramp anyway): the first sub starts ~1.5us
                # earlier than behind the 2.5us ACT cast
                nc.vector.tensor_copy(pred_t[:, 0:HF], p_stage[:, 0:HF])
            else:
                nc.scalar.activation(
                    pred_t[:, i * HF : (i + 1) * HF],
                    p_stage[:, i * HF : (i + 1) * HF],
                    Copy,
                )

        # ---- partition-shifted gt copies, built ON-CHIP by the (idle)
        # TensorEngine: matmul with a shifted identity moves partition
        # p+k -> p exactly (bf16 x {0,1} is lossless), landing in PSUM
        # f32; one ACT op casts PSUM -> bf16 SBUF.  SBUF->SBUF DMA
        # measured 5-23 GB/s (60us+ per shift) so DMA is not an option.
        # The base tile's PAD_BIG margins shift along with the data; the
        # boundary partition (no source row) comes out 0 and is patched
        # to PAD_BIG by a tiny one-partition DMA from a const tile. ----
        WTOT = MARG + FDW + MARG
        eye_t = g_pool.tile([128, n_eyes * 128], f16, tag="eye", name="eye_t")
        nc.scalar.dma_start(eye_t[:, :], eye_d.ap())
        cpad = g_pool.tile([32, WTOT], f16, tag="cpad", name="cpad")
        nc.gpsimd.memset(cpad[:, :], PAD_BIG)
        ps_pool = ctx.enter_context(tc.tile_pool(name="ps", bufs=1, space="PSUM"))
        for ei, k in enumerate([k for k in ks_needed if k != 0]):
            t = g_pool.tile([128, WTOT], f16, tag=f"gt{k}", name=f"gt{k}")
            ps = ps_pool.tile([128, WTOT], f32, tag="ps", name=f"ps{k}")
            lhsT = eye_t[:, ei * 128 : (ei + 1) * 128]
            for c in range(0, WTOT, 512):
                wid = min(512, WTOT - c)
                nc.tensor.matmul(
                    ps[:, c : c + wid],
                    lhsT,
                    gt_t[0][:, c : c + wid],
                    start=True,
                    stop=True,
                )
            nc.scalar.activation(t[:, :], ps[:, :], Copy)
            bp = 0 if k < 0 else 127
            nc.sync.dma_start(t[bp : bp + 1, :], cpad[0:1, :])
            gt_t[k] = t
        nc._shift_ks = [k for k in ks_needed if k != 0]

        m = None
        pair_sG = {}
        done = set()
        for ui, (di, s) in enumerate(units):
            k, sp = shift_of(di, s)

            # ---- wide sub: all G column shifts in one 2x bf16 DVE op.
            # The first two units sub per-q-chunk, matching the three
            # base-load dmas, so DVE starts on the first channel early ----
            d = d_pool.tile([128, GQW], f16, tag="d", name=f"d{di}_{s}")
            isplit = Q_CHUNKS if ui < 2 else [(0, Q)]
            for q0, q1 in isplit:
                nq = q1 - q0
                gt_op = strided(
                    gt_t[k][:, :],
                    [[1, G], [SW, nq], [1, W]],
                    MARG + sp * W - W_PAD + q0 * SW,
                )
                pr_op = strided(
                    pred_t[:, :], [[0, G], [SW, nq], [1, W]], s * W + q0 * SW
                )
                d_out = strided(
                    d[:, :], [[Q * W, G], [W, nq], [1, W]], q0 * W
                )
                nc.vector.tensor_sub(d_out, gt_op, pr_op)

            # ---- |d| in place on ACT ----
            nc.scalar.activation(d[:, :], d[:, :], Abs)

            # ---- channel sum: q = i*C + c, so c-slices are strided views
            CW = C * W
            dc = [
                strided(d[:, :], [[Q * W, G], [CW, IPC], [1, W]], c * W)
                for c in range(C)
            ]
            s01 = s_pool.tile([128, GIW], f16, tag="s01", name=f"s01_{di}_{s}")
            v01 = strided(s01[:, :], [[IPC * W, G], [W, IPC], [1, W]])
            nc.vector.tensor_add(v01, dc[0], dc[1])
            # the channel sum lands in this di's SHARED pair tile
            # [s-half][g][i][w], so each min update covers both sub-rows
            if di not in pair_sG:
                pair_sG[di] = s_pool.tile(
                    [128, S * GIW], f16, tag="sGp", name=f"sGp_{di}"
                )
            sGp = pair_sG[di]
            vG = strided(
                sGp[:, :], [[IPC * W, G], [W, IPC], [1, W]], s * GIW
            )
            nc.vector.tensor_add(vG, v01, dc[2])

            done.add((di, s))
            if (di, 1 - s) not in done:
                continue

            # ---- both sub-rows ready: w-restricted running-min updates
            # into the shared m [i][s][w], one op per (di, g) ----
            def sview(g, lo, hi):
                return strided(
                    sGp[:, :], [[W, IPC], [GIW, S], [1, hi - lo]],
                    g * IPC * W + lo,
                )

            if m is None:
                # init from this pair's center column shift: always
                # w-valid, and row-invalid entries hold PAD_BIG which
                # later min updates displace
                m = m_pool.tile([128, IW], f16, tag="m", name="m")
                nc.scalar.activation(
                    strided(m[:, :], [[SW, IPC], [W, S], [1, W]]),
                    sview(W_PAD, 0, W),
                    Copy,
                )
                order = [g for g in range(G) if g != W_PAD]
            else:
                order = list(range(G))
            for g in order:
                lo, hi = wrange(g)
                mv = strided(m[:, :], [[SW, IPC], [W, S], [1, hi - lo]], lo)
                nc.vector.tensor_tensor(mv, mv, sview(g, lo, hi), AluOpType.min)

        # ---- free-dim reduce -> [128,1] fp32 partials, then collapse
        # the partition dim on the PE (ones-vector contraction) so the
        # output dma is ONE 4-byte descriptor: a [128,1] dma is 128
        # 4-byte descriptors whose completion receipt burned ~7us ----
        # (ones/tot/res created last so the tiny allocations don't shift
        # the hot tiles' SBUF addresses -- a 4B insertion mid-sequence
        # measured a uniform ~20% slowdown of every DVE op)
        ones = r_pool.tile([128, 1], f32, tag="ones", name="ones")
        nc.gpsimd.memset(ones[:, :], 1.0)
        tot = r_pool.tile([128, 1], f32, tag="tot", name="tot")
        nc.vector.tensor_reduce(tot, m, mybir.AxisListType.X, AluOpType.add)
        ps_tot = ps_pool.tile([1, 1], f32, tag="ps_tot", name="ps_tot")
        nc.tensor.matmul(ps_tot[:, :], ones[:, :], tot[:, :], start=True, stop=True)
        res = r_pool.tile([1, 1], f32, tag="res", name="res")
        nc.scalar.activation(res[:, :], ps_tot[:, :], Copy)
        nc.sync.dma_start(out_d.ap()[:, :], res[:, :])

    nc.compile()
    return nc


def _get_nc(nh, nw):
    key = (nh, nw)
    if key not in _BUILD_CACHE:
        _BUILD_CACHE[key] = _build(nh, nw)
    return _BUILD_CACHE[key]


def _setup_trace():
    """Register the axon NTFF profile hook (the image's antenv lacks
    axon_hooks) and stub the artifact upload so trace=True works."""
    import sys
    import types

    from concourse import bass_utils

    try:
        import antenv.axon_hooks  # noqa: F401
    except ImportError:
        try:
            import trn_agent_boot.trn_boot as tb

            hook = tb._ntff_profile_via_ctypes("/opt/axon/libaxon_pjrt.so")
            mod = types.ModuleType("antenv.axon_hooks")
            mod.get_axon_ntff_profile_hook = lambda: hook
            sys.modules["antenv.axon_hooks"] = mod
        except Exception as e:  # profiling is best-effort
            print(f"ntff hook setup failed: {e}")
            return False
    bass_utils.upload_artifacts = lambda tmpdir: f"local:{tmpdir}"
    return True


def kernel(predicted, ground_truth, nh=5, nw=5):
    from concourse import bass_utils

    nh, nw = int(nh), int(nw)
    pred = np.ascontiguousarray(np.asarray(predicted, dtype=np.float32))
    gt = np.ascontiguousarray(np.asarray(ground_truth, dtype=np.float32))
    assert pred.shape == (B, C, H, W) and gt.shape == (B, C, H, W)

    nc = _get_nc(nh, nw)
    import ml_dtypes

    eye = np.concatenate(
        [np.eye(128, k=-k) for k in nc._shift_ks], axis=1
    ).astype(ml_dtypes.bfloat16)
    in_maps = [
        {
            "predicted": pred[k * IPC : (k + 1) * IPC],
            "ground_truth": gt[k * IPC : (k + 1) * IPC],
            "shifteye": eye,
        }
        for k in range(N_CORES)
    ]
    trace = bool(int(os.environ.get("NNLOSS_TRACE", "0")))
    if trace:
        trace = _setup_trace()
    res = bass_utils.run_bass_kernel_spmd(
        nc, in_maps, list(range(N_CORES)), trace=trace
    )
    LAST_EXEC_NS[0] = res.exec_time_ns
    LAST_RES[0] = res
    total = 0.0
    for r in res.results:
        total += float(np.asarray(r["partials"], dtype=np.float64).sum())
    return np.float32(total / (B * H * W))
